# revision 20
# baseline (speedup 1.0000x reference)
"""Trainium2 Bass kernel for a capacity-based MoE router (8 NeuronCores).

Reference computation:
    logits = x @ gate_weight.T ; softmax ; top-2 (renormalized)
    per-expert capacity: keep top-1024 assignments by prob (stable ties by
    flat index); aux lb/z losses and per-expert usage counts.

Sharding: x token-sharded across 8 cores, supplied pre-transposed
[4096, 1024] per core so the contraction dim lands on SBUF partitions with
contiguous DMA; gate weight replicated (pre-packed [128, 32*8]). Each core:
logits via 64 fp32 PE matmuls (W stationary, tokens moving), vectorized
softmax/top-2, AllGather of (prob, expert) planes, then core c does the
exact global capacity selection for expert c: a radix threshold search on
fp32 bit patterns (3 bits/round, bitwise-OR candidate assembly — DVE int
adds run through the f32 pipeline and round, ORs are lane-exact) plus an
exact stable tie-break prefix scan. Host combines shards/keep masks.
"""

import sys
import types

import numpy as np

N_TOKENS = 8192
D_MODEL = 4096
N_EXPERTS = 8
TOP_K = 2
N_CORES = 8
TPC = N_TOKENS // N_CORES          # tokens per core
N_BANDS = TPC // 128
N_CHUNKS = D_MODEL // 128
CAPACITY = N_TOKENS // N_EXPERTS
LOCAL_NK = TPC * TOP_K
LB_WEIGHT = 0.01
Z_WEIGHT = 0.001
DIGIT_BITS = 3
N_DIGITS = 10  # 30 bits of fp32 bit-pattern space


def _ensure_axon_hooks():
    """Provide antenv.axon_hooks (NTFF profiling plumbing) if the image lacks it."""
    try:
        import antenv.axon_hooks  # noqa: F401
        return
    except ImportError:
        pass
    try:
        import antenv
    except ImportError:
        return
    m = types.ModuleType("antenv.axon_hooks")
    m._hook = None
    m.set_axon_ntff_profile_hook = lambda h: setattr(m, "_hook", h)
    m.get_axon_ntff_profile_hook = lambda: m._hook
    sys.modules["antenv.axon_hooks"] = m
    antenv.axon_hooks = m
    try:
        from trn_agent_boot.trn_boot import _ntff_profile_via_ctypes

        m.set_axon_ntff_profile_hook(
            _ntff_profile_via_ctypes("/opt/axon/libaxon_pjrt.so")
        )
    except Exception:
        pass


_ensure_axon_hooks()

import concourse.bacc as bacc  # noqa: E402
import concourse.bass_utils as bass_utils  # noqa: E402
import concourse.mybir as mybir  # noqa: E402
import concourse.tile as tile  # noqa: E402
from concourse.ap import AP  # noqa: E402

bass_utils.upload_artifacts = lambda tmpdir: f"local://{tmpdir}"

F32 = mybir.dt.float32
I32 = mybir.dt.int32
U32 = mybir.dt.uint32
Alu = mybir.AluOpType
Act = mybir.ActivationFunctionType
AX = mybir.AxisListType


def _body(nc, tc, xt, wt, cid, payload_out, keep_out, stats_out):
    with (
        tc.tile_pool(name="const", bufs=1) as cp,
        tc.tile_pool(name="xb", bufs=3) as xbp,
        tc.tile_pool(name="ep", bufs=1) as ep,
        tc.tile_pool(name="sel", bufs=1) as selp,
        tc.tile_pool(name="srch", bufs=2) as srp,
        tc.tile_pool(name="psA", bufs=1, space="PSUM") as psA,
        tc.tile_pool(name="pss", bufs=1, space="PSUM") as pss,
        tc.tile_pool(name="dram", bufs=1, space="DRAM") as dramp,
    ):
        # ---------------- constants ----------------
        wt_sb = cp.tile([128, N_CHUNKS, N_EXPERTS], F32)
        nc.sync.dma_start(wt_sb[:], wt.rearrange("p (c e) -> p c e", e=N_EXPERTS))
        cid_sb = cp.tile([128, 1], F32)
        nc.sync.dma_start(cid_sb[:], cid[:])
        ones_col = cp.tile([128, 1], F32)
        nc.vector.memset(ones_col[:], 1.0)
        ones_mat = cp.tile([128, 128], F32)
        nc.vector.memset(ones_mat[:], 1.0)
        lmat = cp.tile([128, 128], F32)
        nc.gpsimd.affine_select(
            lmat[:], ones_mat[:], pattern=[[1, 128]], compare_op=Alu.is_gt,
            fill=0.0, base=0, channel_multiplier=-1,
        )
        ident8 = cp.tile([8, 8], F32)
        nc.gpsimd.affine_select(
            ident8[:], ones_mat[0:8, 0:8], pattern=[[1, 8]],
            compare_op=Alu.is_equal, fill=0.0, base=0, channel_multiplier=-1,
        )
        eio_i = cp.tile([128, N_BANDS, N_EXPERTS], I32)
        nc.gpsimd.iota(
            eio_i[:], pattern=[[0, N_BANDS], [1, N_EXPERTS]], base=0,
            channel_multiplier=0,
        )
        eio = cp.tile([128, N_BANDS, N_EXPERTS], F32)
        nc.vector.tensor_copy(eio[:], eio_i[:])

        # ---------------- logits: logitsT[e, t] accumulated over 32 chunks ----
        pA = [
            psA.tile([N_EXPERTS, 512], F32, tag=f"pA{h}", name=f"pA{h}")
            for h in range(2)
        ]
        for cc in range(N_CHUNKS // 2):
            xb = xbp.tile([128, 2, TPC], F32, tag="xb")
            nc.sync.dma_start(
                xb[:],
                xt[cc * 256 : (cc + 1) * 256, :].rearrange(
                    "(two p) t -> p two t", p=128
                ),
            )
            for sub in range(2):
                c = cc * 2 + sub
                for h in range(2):
                    nc.tensor.matmul(
                        pA[h][:], wt_sb[:, c, :],
                        xb[:, sub, h * 512 : (h + 1) * 512],
                        start=(c == 0), stop=(c == N_CHUNKS - 1),
                    )
        lgT = ep.tile([N_EXPERTS, TPC], F32)
        for h in range(2):
            nc.scalar.copy(lgT[:, h * 512 : (h + 1) * 512], pA[h][:])
        # transpose to [tok, e] per band
        pst = pss.tile([128, N_BANDS * N_EXPERTS], F32, tag="pst")
        for b in range(N_BANDS):
            nc.tensor.transpose(
                pst[:, b * 8 : (b + 1) * 8],
                lgT[:, b * 128 : (b + 1) * 128],
                ident8[:],
            )
        lg3 = ep.tile([128, N_BANDS, N_EXPERTS], F32)
        nc.vector.tensor_copy(lg3[:], pst[:].rearrange("p (b e) -> p b e", e=8))

        # ---------------- vectorized per-token routing ----------------
        def bc(t8):  # [128, 8] -> broadcast [128, 8, 8] along expert dim
            return t8[:].unsqueeze(2).broadcast_to((128, N_BANDS, N_EXPERTS))

        mx8 = ep.tile([128, N_BANDS], F32)
        nc.vector.reduce_max(mx8[:], lg3[:], axis=AX.X)
        lc3 = ep.tile([128, N_BANDS, N_EXPERTS], F32)
        nc.vector.tensor_tensor(lc3[:], lg3[:], bc(mx8), op=Alu.subtract)
        ex3 = ep.tile([128, N_BANDS, N_EXPERTS], F32)
        nc.scalar.activation(ex3[:], lc3[:], Act.Exp)
        sumex = ep.tile([128, N_BANDS], F32)
        nc.vector.reduce_sum(sumex[:], ex3[:], axis=AX.X)
        # z partials: (mx + ln(sumex))^2
        lse = ep.tile([128, N_BANDS], F32)
        nc.scalar.activation(lse[:], sumex[:], Act.Ln)
        lset = ep.tile([128, N_BANDS], F32)
        nc.vector.tensor_tensor(lset[:], lse[:], mx8[:], op=Alu.add)
        zsq = ep.tile([128, N_BANDS], F32)
        nc.scalar.activation(zsq[:], lset[:], Act.Square)
        # router prob column sums (over bands) -> [128, 8e]
        rs8 = ep.tile([128, N_BANDS], F32)
        nc.vector.reciprocal(rs8[:], sumex[:])
        probs3 = ep.tile([128, N_BANDS, N_EXPERTS], F32)
        nc.vector.tensor_tensor(probs3[:], ex3[:], bc(rs8), op=Alu.mult)
        colacc = ep.tile([128, N_EXPERTS], F32)
        nc.vector.reduce_sum(
            colacc[:], probs3[:].transpose([0, 2, 1]), axis=AX.X
        )
        # top-2 (first-index tie semantics)
        eq1 = ep.tile([128, N_BANDS, N_EXPERTS], F32)
        nc.vector.tensor_tensor(eq1[:], lg3[:], bc(mx8), op=Alu.is_ge)
        i1c = ep.tile([128, N_BANDS, N_EXPERTS], F32)
        nc.vector.scalar_tensor_tensor(
            i1c[:], in0=eio[:], scalar=9.0, in1=eq1[:], op0=Alu.subtract,
            op1=Alu.mult,
        )  # (e - 9) * eq1 : 0 when not max, e-9 when max
        idx1 = ep.tile([128, N_BANDS], F32)
        nc.vector.tensor_reduce(idx1[:], i1c[:], axis=AX.X, op=Alu.min)
        nc.vector.tensor_scalar_add(idx1[:], idx1[:], 9.0)
        # mask out ONLY the first (lowest-index) max occurrence
        eqf = ep.tile([128, N_BANDS, N_EXPERTS], F32)
        nc.vector.tensor_tensor(eqf[:], eio[:], bc(idx1), op=Alu.is_equal)
        lg2 = ep.tile([128, N_BANDS, N_EXPERTS], F32)
        nc.vector.scalar_tensor_tensor(
            lg2[:], in0=eqf[:], scalar=-1e30, in1=lg3[:], op0=Alu.mult,
            op1=Alu.add,
        )
        v2 = ep.tile([128, N_BANDS], F32)
        nc.vector.reduce_max(v2[:], lg2[:], axis=AX.X)
        eq2 = ep.tile([128, N_BANDS, N_EXPERTS], F32)
        nc.vector.tensor_tensor(eq2[:], lg2[:], bc(v2), op=Alu.is_ge)
        i2c = ep.tile([128, N_BANDS, N_EXPERTS], F32)
        nc.vector.scalar_tensor_tensor(
            i2c[:], in0=eio[:], scalar=9.0, in1=eq2[:], op0=Alu.subtract,
            op1=Alu.mult,
        )
        idx2 = ep.tile([128, N_BANDS], F32)
        nc.vector.tensor_reduce(idx2[:], i2c[:], axis=AX.X, op=Alu.min)
        nc.vector.tensor_scalar_add(idx2[:], idx2[:], 9.0)
        # renormalized top-2 softmax
        d8 = ep.tile([128, N_BANDS], F32)
        nc.vector.tensor_tensor(d8[:], v2[:], mx8[:], op=Alu.subtract)
        ed8 = ep.tile([128, N_BANDS], F32)
        nc.scalar.activation(ed8[:], d8[:], Act.Exp)
        sm8 = ep.tile([128, N_BANDS], F32)
        nc.vector.tensor_scalar_add(sm8[:], ed8[:], 1.0)
        r8 = ep.tile([128, N_BANDS], F32)
        nc.vector.reciprocal(r8[:], sm8[:])
        p2_8 = ep.tile([128, N_BANDS], F32)
        nc.vector.tensor_tensor(p2_8[:], ed8[:], r8[:], op=Alu.mult)
        s2_8 = ep.tile([128, N_BANDS], F32)
        nc.vector.tensor_tensor(s2_8[:], r8[:], p2_8[:], op=Alu.add)
        s2c8 = ep.tile([128, N_BANDS], F32)
        nc.vector.tensor_scalar_max(s2c8[:], s2_8[:], 1e-8)
        r2_8 = ep.tile([128, N_BANDS], F32)
        nc.vector.reciprocal(r2_8[:], s2c8[:])
        pay_p = cp.tile([128, 2 * N_BANDS], F32)
        pay_i = cp.tile([128, 2 * N_BANDS], F32)
        pay_p2 = pay_p[:].rearrange("p (b k) -> p b k", k=2)
        pay_i2 = pay_i[:].rearrange("p (b k) -> p b k", k=2)
        nc.vector.tensor_tensor(pay_p2[:, :, 0], r8[:], r2_8[:], op=Alu.mult)
        nc.vector.tensor_tensor(pay_p2[:, :, 1], p2_8[:], r2_8[:], op=Alu.mult)
        nc.vector.tensor_copy(pay_i2[:, :, 0], idx1[:])
        nc.vector.tensor_copy(pay_i2[:, :, 1], idx2[:])

        # ---------------- exchange (flat order j = 2*t + k) ----------------
        gin = dramp.tile([2, LOCAL_NK], F32)
        gout = dramp.tile([N_CORES, 2, LOCAL_NK], F32)

        def j_order_ap(dram_ap, row):
            base = dram_ap[row]
            return AP(base.tensor, base.offset, [[2, 128], [256, 8], [1, 2]])

        pay_p3 = pay_p[:].rearrange("p (b k) -> p b k", k=2)
        pay_i3 = pay_i[:].rearrange("p (b k) -> p b k", k=2)
        nc.sync.dma_start(j_order_ap(gin, 0), pay_p3)
        nc.sync.dma_start(j_order_ap(gin, 1), pay_i3)
        nc.sync.dma_start(j_order_ap(payload_out, 0), pay_p3)
        nc.sync.dma_start(j_order_ap(payload_out, 1), pay_i3)
        nc.gpsimd.collective_compute(
            "AllGather",
            Alu.bypass,
            replica_groups=[list(range(N_CORES))],
            ins=[gin[:].opt()],
            outs=[gout[:].opt()],
        )

        def plane_ap(plane):
            base = gout[0, plane]
            return AP(base.tensor, base.offset, [[4096, 8], [128, 16], [1, 128]])

        pf = selp.tile([128, 128], F32)
        nc.sync.dma_start(pf[:], plane_ap(0))
        idxf = selp.tile([128, 128], F32)
        nc.sync.dma_start(idxf[:], plane_ap(1))

        # ---------------- capacity selection for expert == core id ----------
        m = selp.tile([128, 128], mybir.dt.uint8)
        nc.vector.tensor_scalar(m[:], idxf[:], cid_sb[:], None, op0=Alu.is_equal)
        # masked keys: non-expert slots become -1.0, which every candidate
        # threshold (>= +0.0) excludes automatically
        pkey = selp.tile([128, 128], F32)
        nc.vector.memset(pkey[:], -1.0)
        nc.vector.copy_predicated(pkey[:], m[:], pf[:])

        tau = selp.tile([128, 1], I32)
        nc.vector.memset(tau[:], 0)
        ncand = (1 << DIGIT_BITS) - 1
        for dgt in range(N_DIGITS - 1, -1, -1):
            base = 1 << (DIGIT_BITS * dgt)
            cnt = srp.tile([128, ncand], F32, tag="cnt")
            for j in range(1, ncand + 1):
                cand = srp.tile([128, 1], I32, tag=f"cand{j}")
                nc.vector.tensor_scalar(
                    cand[:], tau[:], j * base, None, op0=Alu.bitwise_or
                )
                scr = srp.tile([128, 128], F32, tag=f"scr{j}")
                nc.vector.tensor_scalar(
                    scr[:], pkey[:], cand[:].bitcast(F32), None,
                    op0=Alu.is_ge, op1=Alu.add, accum_out=cnt[:, j - 1 : j],
                )
            tot = pss.tile([128, ncand], F32, tag="tot")
            nc.tensor.matmul(tot[:], ones_mat[:], cnt[:], start=True, stop=True)
            dscr = srp.tile([128, ncand], F32, tag="dscr")
            digit = srp.tile([128, 1], F32, tag="digit")
            nc.vector.tensor_scalar(
                dscr[:], tot[:], float(CAPACITY), None, op0=Alu.is_ge,
                op1=Alu.add, accum_out=digit[:],
            )
            inc = srp.tile([128, 1], I32, tag="inc")
            nc.vector.tensor_scalar(inc[:], digit[:], float(base), None, op0=Alu.mult)
            nc.vector.tensor_tensor(tau[:], tau[:], inc[:], op=Alu.bitwise_or)

        # ---------------- exact stable tie-break ----------------
        mge = selp.tile([128, 128], F32)
        s2col = selp.tile([128, 2], F32)
        nc.vector.tensor_scalar(
            mge[:], pkey[:], tau[:].bitcast(F32), None,
            op0=Alu.is_ge, op1=Alu.add, accum_out=s2col[:, 0:1],
        )
        meq = selp.tile([128, 128], F32)
        nc.vector.tensor_scalar(
            meq[:], pkey[:], tau[:].bitcast(F32), None,
            op0=Alu.is_equal, op1=Alu.add, accum_out=s2col[:, 1:2],
        )
        tot2 = pss.tile([128, 2], F32, tag="tot2")
        nc.tensor.matmul(tot2[:], ones_mat[:], s2col[:], start=True, stop=True)
        tot2_sb = selp.tile([128, 2], F32)
        nc.vector.tensor_copy(tot2_sb[:], tot2[:])
        budget = selp.tile([128, 1], F32)
        # budget = CAP - (tot_ge - tot_eq)
        nc.vector.scalar_tensor_tensor(
            budget[:], in0=tot2_sb[:, 1:2], scalar=float(CAPACITY), in1=tot2_sb[:, 0:1],
            op0=Alu.add, op1=Alu.subtract,
        )
        pres = selp.tile([128, 128], F32)
        nc.vector.tensor_tensor_scan(
            pres[:], meq[:], meq[:], 0.0, op0=Alu.add, op1=Alu.bypass
        )
        offs = pss.tile([128, 1], F32, tag="offs")
        nc.tensor.matmul(offs[:], lmat[:], pres[:, 127:128], start=True, stop=True)
        offs_sb = selp.tile([128, 1], F32)
        nc.vector.tensor_copy(offs_sb[:], offs[:])
        excl = selp.tile([128, 128], F32)
        nc.vector.scalar_tensor_tensor(
            excl[:], in0=pres[:], scalar=offs_sb[:], in1=meq[:],
            op0=Alu.add, op1=Alu.subtract,
        )
        tieok = selp.tile([128, 128], F32)
        nc.vector.tensor_scalar(tieok[:], excl[:], budget[:], None, op0=Alu.is_lt)
        tkeep = selp.tile([128, 128], F32)
        nc.vector.tensor_tensor(tkeep[:], meq[:], tieok[:], op=Alu.mult)
        keep = selp.tile([128, 128], F32)
        nc.vector.tensor_tensor(keep[:], mge[:], meq[:], op=Alu.subtract)
        nc.vector.tensor_tensor(keep[:], keep[:], tkeep[:], op=Alu.add)
        nc.sync.dma_start(keep_out[:], keep[:])

        # ---------------- stats ----------------
        colsum_ps = pss.tile([N_EXPERTS, 1], F32, tag="colsum")
        nc.tensor.matmul(colsum_ps[:], colacc[:], ones_col[:], start=True, stop=True)
        colsum_sb = selp.tile([N_EXPERTS, 1], F32)
        nc.vector.tensor_copy(colsum_sb[:], colsum_ps[:])
        zrow = selp.tile([128, 1], F32)
        nc.vector.reduce_sum(zrow[:], zsq[:], axis=AX.X)
        zps = pss.tile([1, 1], F32, tag="zps")
        nc.tensor.matmul(zps[:], zrow[:], ones_col[:], start=True, stop=True)
        zsb = selp.tile([1, 1], F32)
        nc.vector.tensor_copy(zsb[:], zps[:])
        nc.sync.dma_start(
            stats_out[0:N_EXPERTS].rearrange("(p o) -> p o", o=1), colsum_sb[:]
        )
        nc.sync.dma_start(
            stats_out[N_EXPERTS : N_EXPERTS + 1].rearrange("(p o) -> p o", o=1),
            zsb[:],
        )


def build_program():
    nc = bacc.Bacc(
        "TRN2",
        target_bir_lowering=False,
        debug=False,
        enable_asserts=False,
        num_devices=N_CORES,
    )
    xt = nc.dram_tensor("xt", [D_MODEL, TPC], F32, kind="ExternalInput").ap()
    wt = nc.dram_tensor(
        "wt", [128, N_CHUNKS * N_EXPERTS], F32, kind="ExternalInput"
    ).ap()
    cid = nc.dram_tensor("cid", [128, 1], F32, kind="ExternalInput").ap()
    payload_out = nc.dram_tensor(
        "payload", [2, LOCAL_NK], F32, kind="ExternalOutput"
    ).ap()
    keep_out = nc.dram_tensor("keep", [128, 128], F32, kind="ExternalOutput").ap()
    stats_out = nc.dram_tensor("stats", [16], F32, kind="ExternalOutput").ap()

    with tile.TileContext(nc) as tc:
        _body(nc, tc, xt, wt, cid, payload_out, keep_out, stats_out)
    nc.compile()
    return nc


_PROGRAM = None


def _get_program():
    global _PROGRAM
    if _PROGRAM is None:
        _PROGRAM = build_program()
    return _PROGRAM


def make_in_maps(x, gate_weight):
    x = np.asarray(x, dtype=np.float32)
    gw = np.asarray(gate_weight, dtype=np.float32)
    # wt[p, c*8 + e] = gw[e, c*128 + p]
    wt = np.ascontiguousarray(
        gw.T.reshape(N_CHUNKS, 128, N_EXPERTS).transpose(1, 0, 2).reshape(128, -1)
    )
    in_maps = []
    for c in range(N_CORES):
        in_maps.append(
            {
                "xt": np.ascontiguousarray(x[c * TPC : (c + 1) * TPC, :].T),
                "wt": wt,
                "cid": np.full((128, 1), float(c), np.float32),
            }
        )
    return in_maps


def assemble_outputs(per_core):
    """per_core: list of dicts with payload [2,2048], keep [128,128], stats [16]."""
    idx_all = np.empty((N_TOKENS, TOP_K), np.int32)
    p_all = np.empty((N_TOKENS, TOP_K), np.float32)
    keep_planes = []
    colsum = np.zeros(N_EXPERTS, np.float64)
    zsum = 0.0
    for c in range(N_CORES):
        r = per_core[c]
        pay = r["payload"]
        p_all[c * TPC : (c + 1) * TPC] = pay[0].reshape(TPC, TOP_K)
        idx_all[c * TPC : (c + 1) * TPC] = np.rint(
            pay[1].reshape(TPC, TOP_K)
        ).astype(np.int32)
        keep_planes.append(r["keep"].reshape(-1))
        colsum += r["stats"][0:N_EXPERTS].astype(np.float64)
        zsum += float(r["stats"][N_EXPERTS])
    keep_flat = np.zeros(N_TOKENS * TOP_K, np.float32)
    for kp in keep_planes:
        keep_flat += kp
    keep = (keep_flat > 0.5).reshape(N_TOKENS, TOP_K)
    final_idx = np.where(keep, idx_all, -1).astype(np.int32)
    final_p = np.where(keep, p_all, 0.0).astype(np.float32)
    usage = np.array([kp[0::2].sum() for kp in keep_planes], np.float32)
    lb_loss = np.float32(
        (colsum * usage.astype(np.float64)).sum() * LB_WEIGHT / (N_TOKENS * N_EXPERTS)
    )
    z_loss = np.float32(zsum / N_TOKENS * Z_WEIGHT)
    return final_idx, final_p, lb_loss, z_loss, usage


def run_device(x, gate_weight, trace=False, **kwargs):
    nc = _get_program()
    in_maps = make_in_maps(x, gate_weight)
    res = bass_utils.run_bass_kernel_spmd(
        nc, in_maps, core_ids=list(range(N_CORES)), trace=trace, **kwargs
    )
    return res


def kernel(x, gate_weight):
    res = run_device(x, gate_weight, trace=False)
    return assemble_outputs(res.results)


# revision 22
# speedup vs baseline: 1.1926x; 1.1926x over previous
"""Trainium2 Bass kernel for a capacity-based MoE router (8 NeuronCores).

Reference computation:
    logits = x @ gate_weight.T ; softmax ; top-2 (renormalized)
    per-expert capacity: keep top-1024 assignments by prob (stable ties by
    flat index); aux lb/z losses and per-expert usage counts.

Sharding: x token-sharded across 8 cores, supplied pre-transposed
[4096, 1024] per core so the contraction dim lands on SBUF partitions with
contiguous DMA; gate weight replicated (pre-packed [128, 32*8]). Each core:
logits via 64 fp32 PE matmuls (W stationary, tokens moving), vectorized
softmax/top-2, AllGather of (prob, expert) planes, then core c does the
exact global capacity selection for expert c: a radix threshold search on
fp32 bit patterns (3 bits/round, bitwise-OR candidate assembly — DVE int
adds run through the f32 pipeline and round, ORs are lane-exact) plus an
exact stable tie-break prefix scan. Host combines shards/keep masks.
"""

import sys
import types

import numpy as np

N_TOKENS = 8192
D_MODEL = 4096
N_EXPERTS = 8
TOP_K = 2
N_CORES = 8
TPC = N_TOKENS // N_CORES          # tokens per core
N_BANDS = TPC // 128
N_CHUNKS = D_MODEL // 128
CAPACITY = N_TOKENS // N_EXPERTS
LOCAL_NK = TPC * TOP_K
LB_WEIGHT = 0.01
Z_WEIGHT = 0.001
DIGIT_BITS = 3
N_DIGITS = 10  # 30 bits of fp32 bit-pattern space


def _ensure_axon_hooks():
    """Provide antenv.axon_hooks (NTFF profiling plumbing) if the image lacks it."""
    try:
        import antenv.axon_hooks  # noqa: F401
        return
    except ImportError:
        pass
    try:
        import antenv
    except ImportError:
        return
    m = types.ModuleType("antenv.axon_hooks")
    m._hook = None
    m.set_axon_ntff_profile_hook = lambda h: setattr(m, "_hook", h)
    m.get_axon_ntff_profile_hook = lambda: m._hook
    sys.modules["antenv.axon_hooks"] = m
    antenv.axon_hooks = m
    try:
        from trn_agent_boot.trn_boot import _ntff_profile_via_ctypes

        m.set_axon_ntff_profile_hook(
            _ntff_profile_via_ctypes("/opt/axon/libaxon_pjrt.so")
        )
    except Exception:
        pass


_ensure_axon_hooks()

import concourse.bacc as bacc  # noqa: E402
import concourse.bass_utils as bass_utils  # noqa: E402
import concourse.mybir as mybir  # noqa: E402
import concourse.tile as tile  # noqa: E402
from concourse.ap import AP  # noqa: E402

bass_utils.upload_artifacts = lambda tmpdir: f"local://{tmpdir}"

F32 = mybir.dt.float32
I32 = mybir.dt.int32
U32 = mybir.dt.uint32
Alu = mybir.AluOpType
Act = mybir.ActivationFunctionType
AX = mybir.AxisListType


def _body(nc, tc, xt, wt, cid, payload_out, keep_out, stats_out):
    with (
        tc.tile_pool(name="const", bufs=1) as cp,
        tc.tile_pool(name="xb", bufs=3) as xbp,
        tc.tile_pool(name="ep", bufs=1) as ep,
        tc.tile_pool(name="sel", bufs=1) as selp,
        tc.tile_pool(name="srch", bufs=2) as srp,
        tc.tile_pool(name="psA", bufs=1, space="PSUM") as psA,
        tc.tile_pool(name="pss", bufs=1, space="PSUM") as pss,
        tc.tile_pool(name="dram", bufs=1, space="DRAM") as dramp,
    ):
        # ---------------- constants ----------------
        wt_sb = cp.tile([128, N_CHUNKS, N_EXPERTS], F32)
        nc.sync.dma_start(wt_sb[:], wt.rearrange("p (c e) -> p c e", e=N_EXPERTS))
        cid_sb = cp.tile([128, 1], F32)
        nc.sync.dma_start(cid_sb[:], cid[:])
        ones_col = cp.tile([128, 1], F32)
        nc.vector.memset(ones_col[:], 1.0)
        ones_mat = cp.tile([128, 128], F32)
        nc.vector.memset(ones_mat[:], 1.0)
        lmat = cp.tile([128, 128], F32)
        nc.gpsimd.affine_select(
            lmat[:], ones_mat[:], pattern=[[1, 128]], compare_op=Alu.is_gt,
            fill=0.0, base=0, channel_multiplier=-1,
        )
        ident8 = cp.tile([8, 8], F32)
        nc.gpsimd.affine_select(
            ident8[:], ones_mat[0:8, 0:8], pattern=[[1, 8]],
            compare_op=Alu.is_equal, fill=0.0, base=0, channel_multiplier=-1,
        )
        eio_i = cp.tile([128, N_BANDS, N_EXPERTS], I32)
        nc.gpsimd.iota(
            eio_i[:], pattern=[[0, N_BANDS], [1, N_EXPERTS]], base=0,
            channel_multiplier=0,
        )
        eio = cp.tile([128, N_BANDS, N_EXPERTS], F32)
        nc.vector.tensor_copy(eio[:], eio_i[:])

        # ---------------- logits: logitsT[e, t] accumulated over 32 chunks ----
        pA = [
            psA.tile([N_EXPERTS, 512], F32, tag=f"pA{h}", name=f"pA{h}")
            for h in range(2)
        ]
        for cc in range(N_CHUNKS // 2):
            xb = xbp.tile([128, 2, TPC], F32, tag="xb")
            nc.sync.dma_start(
                xb[:],
                xt[cc * 256 : (cc + 1) * 256, :].rearrange(
                    "(two p) t -> p two t", p=128
                ),
            )
            for sub in range(2):
                c = cc * 2 + sub
                for h in range(2):
                    nc.tensor.matmul(
                        pA[h][:], wt_sb[:, c, :],
                        xb[:, sub, h * 512 : (h + 1) * 512],
                        start=(c == 0), stop=(c == N_CHUNKS - 1),
                    )
        lgT = ep.tile([N_EXPERTS, TPC], F32)
        for h in range(2):
            nc.scalar.copy(lgT[:, h * 512 : (h + 1) * 512], pA[h][:])
        # transpose to [tok, e] per band
        pst = pss.tile([128, N_BANDS * N_EXPERTS], F32, tag="pst")
        for b in range(N_BANDS):
            nc.tensor.transpose(
                pst[:, b * 8 : (b + 1) * 8],
                lgT[:, b * 128 : (b + 1) * 128],
                ident8[:],
            )
        lg3 = ep.tile([128, N_BANDS, N_EXPERTS], F32)
        nc.vector.tensor_copy(lg3[:], pst[:].rearrange("p (b e) -> p b e", e=8))

        # ---------------- vectorized per-token routing ----------------
        def bc(t8):  # [128, 8] -> broadcast [128, 8, 8] along expert dim
            return t8[:].unsqueeze(2).broadcast_to((128, N_BANDS, N_EXPERTS))

        mx8 = ep.tile([128, N_BANDS], F32)
        nc.vector.reduce_max(mx8[:], lg3[:], axis=AX.X)
        lc3 = ep.tile([128, N_BANDS, N_EXPERTS], F32)
        nc.vector.tensor_tensor(lc3[:], lg3[:], bc(mx8), op=Alu.subtract)
        ex3 = ep.tile([128, N_BANDS, N_EXPERTS], F32)
        nc.scalar.activation(ex3[:], lc3[:], Act.Exp)
        sumex = ep.tile([128, N_BANDS], F32)
        nc.vector.reduce_sum(sumex[:], ex3[:], axis=AX.X)
        # z partials: (mx + ln(sumex))^2
        lse = ep.tile([128, N_BANDS], F32)
        nc.scalar.activation(lse[:], sumex[:], Act.Ln)
        lset = ep.tile([128, N_BANDS], F32)
        nc.vector.tensor_tensor(lset[:], lse[:], mx8[:], op=Alu.add)
        zsq = ep.tile([128, N_BANDS], F32)
        nc.scalar.activation(zsq[:], lset[:], Act.Square)
        # router prob column sums (over bands) -> [128, 8e]
        rs8 = ep.tile([128, N_BANDS], F32)
        nc.vector.reciprocal(rs8[:], sumex[:])
        probs3 = ep.tile([128, N_BANDS, N_EXPERTS], F32)
        nc.vector.tensor_tensor(probs3[:], ex3[:], bc(rs8), op=Alu.mult)
        colacc = ep.tile([128, N_EXPERTS], F32)
        nc.vector.reduce_sum(
            colacc[:], probs3[:].transpose([0, 2, 1]), axis=AX.X
        )
        # top-2 (first-index tie semantics)
        eq1 = ep.tile([128, N_BANDS, N_EXPERTS], F32)
        nc.vector.tensor_tensor(eq1[:], lg3[:], bc(mx8), op=Alu.is_ge)
        i1c = ep.tile([128, N_BANDS, N_EXPERTS], F32)
        nc.vector.scalar_tensor_tensor(
            i1c[:], in0=eio[:], scalar=9.0, in1=eq1[:], op0=Alu.subtract,
            op1=Alu.mult,
        )  # (e - 9) * eq1 : 0 when not max, e-9 when max
        idx1 = ep.tile([128, N_BANDS], F32)
        nc.vector.tensor_reduce(idx1[:], i1c[:], axis=AX.X, op=Alu.min)
        nc.vector.tensor_scalar_add(idx1[:], idx1[:], 9.0)
        # mask out ONLY the first (lowest-index) max occurrence
        eqf = ep.tile([128, N_BANDS, N_EXPERTS], F32)
        nc.vector.tensor_tensor(eqf[:], eio[:], bc(idx1), op=Alu.is_equal)
        lg2 = ep.tile([128, N_BANDS, N_EXPERTS], F32)
        nc.vector.scalar_tensor_tensor(
            lg2[:], in0=eqf[:], scalar=-1e30, in1=lg3[:], op0=Alu.mult,
            op1=Alu.add,
        )
        v2 = ep.tile([128, N_BANDS], F32)
        nc.vector.reduce_max(v2[:], lg2[:], axis=AX.X)
        eq2 = ep.tile([128, N_BANDS, N_EXPERTS], F32)
        nc.vector.tensor_tensor(eq2[:], lg2[:], bc(v2), op=Alu.is_ge)
        i2c = ep.tile([128, N_BANDS, N_EXPERTS], F32)
        nc.vector.scalar_tensor_tensor(
            i2c[:], in0=eio[:], scalar=9.0, in1=eq2[:], op0=Alu.subtract,
            op1=Alu.mult,
        )
        idx2 = ep.tile([128, N_BANDS], F32)
        nc.vector.tensor_reduce(idx2[:], i2c[:], axis=AX.X, op=Alu.min)
        nc.vector.tensor_scalar_add(idx2[:], idx2[:], 9.0)
        # renormalized top-2 softmax
        d8 = ep.tile([128, N_BANDS], F32)
        nc.vector.tensor_tensor(d8[:], v2[:], mx8[:], op=Alu.subtract)
        ed8 = ep.tile([128, N_BANDS], F32)
        nc.scalar.activation(ed8[:], d8[:], Act.Exp)
        sm8 = ep.tile([128, N_BANDS], F32)
        nc.vector.tensor_scalar_add(sm8[:], ed8[:], 1.0)
        r8 = ep.tile([128, N_BANDS], F32)
        nc.vector.reciprocal(r8[:], sm8[:])
        p2_8 = ep.tile([128, N_BANDS], F32)
        nc.vector.tensor_tensor(p2_8[:], ed8[:], r8[:], op=Alu.mult)
        s2_8 = ep.tile([128, N_BANDS], F32)
        nc.vector.tensor_tensor(s2_8[:], r8[:], p2_8[:], op=Alu.add)
        s2c8 = ep.tile([128, N_BANDS], F32)
        nc.vector.tensor_scalar_max(s2c8[:], s2_8[:], 1e-8)
        r2_8 = ep.tile([128, N_BANDS], F32)
        nc.vector.reciprocal(r2_8[:], s2c8[:])
        pay_p = cp.tile([128, 2 * N_BANDS], F32)
        pay_i = cp.tile([128, 2 * N_BANDS], F32)
        pay_p2 = pay_p[:].rearrange("p (b k) -> p b k", k=2)
        pay_i2 = pay_i[:].rearrange("p (b k) -> p b k", k=2)
        nc.vector.tensor_tensor(pay_p2[:, :, 0], r8[:], r2_8[:], op=Alu.mult)
        nc.vector.tensor_tensor(pay_p2[:, :, 1], p2_8[:], r2_8[:], op=Alu.mult)
        nc.vector.tensor_copy(pay_i2[:, :, 0], idx1[:])
        nc.vector.tensor_copy(pay_i2[:, :, 1], idx2[:])

        # ---------------- exchange (flat order j = 2*t + k) ----------------
        gin = dramp.tile([2, LOCAL_NK], F32)
        gout = dramp.tile([N_CORES, 2, LOCAL_NK], F32)

        # Host supplies tokens permuted so device slot (band b, partition p)
        # holds host token 8p + b; the payload tile's partition-major layout
        # [p, (b, k)] is then exactly the flat order j = 2*t + k and all
        # bounce writes are contiguous.
        def j_order_ap(dram_ap, row):
            return dram_ap[row].rearrange("(p f) -> p f", p=128)

        nc.sync.dma_start(j_order_ap(gin, 0), pay_p[:])
        nc.sync.dma_start(j_order_ap(gin, 1), pay_i[:])
        nc.sync.dma_start(j_order_ap(payload_out, 0), pay_p[:])
        nc.sync.dma_start(j_order_ap(payload_out, 1), pay_i[:])
        nc.gpsimd.collective_compute(
            "AllGather",
            Alu.bypass,
            replica_groups=[list(range(N_CORES))],
            ins=[gin[:].opt()],
            outs=[gout[:].opt()],
        )

        def plane_ap(plane):
            base = gout[0, plane]
            return AP(base.tensor, base.offset, [[4096, 8], [128, 16], [1, 128]])

        pf = selp.tile([128, 128], F32)
        nc.sync.dma_start(pf[:], plane_ap(0))
        idxf = selp.tile([128, 128], F32)
        nc.sync.dma_start(idxf[:], plane_ap(1))

        # ---------------- capacity selection for expert == core id ----------
        m = selp.tile([128, 128], mybir.dt.uint8)
        nc.vector.tensor_scalar(m[:], idxf[:], cid_sb[:], None, op0=Alu.is_equal)
        # masked keys: non-expert slots become -1.0, which every candidate
        # threshold (>= +0.0) excludes automatically
        pkey = selp.tile([128, 128], F32)
        nc.vector.memset(pkey[:], -1.0)
        nc.vector.copy_predicated(pkey[:], m[:], pf[:])

        tau = selp.tile([128, 1], I32)
        nc.vector.memset(tau[:], 0)
        ncand = (1 << DIGIT_BITS) - 1
        for dgt in range(N_DIGITS - 1, -1, -1):
            base = 1 << (DIGIT_BITS * dgt)
            cnt = srp.tile([128, ncand], F32, tag="cnt")
            for j in range(1, ncand + 1):
                cand = srp.tile([128, 1], I32, tag=f"cand{j}")
                nc.vector.tensor_scalar(
                    cand[:], tau[:], j * base, None, op0=Alu.bitwise_or
                )
                scr = srp.tile([128, 128], F32, tag=f"scr{j}")
                nc.vector.tensor_scalar(
                    scr[:], pkey[:], cand[:].bitcast(F32), None, op0=Alu.is_ge
                )
                nc.vector.reduce_sum(cnt[:, j - 1 : j], scr[:], axis=AX.X)
            tot = pss.tile([128, ncand], F32, tag="tot")
            nc.tensor.matmul(tot[:], ones_mat[:], cnt[:], start=True, stop=True)
            dscr = srp.tile([128, ncand], F32, tag="dscr")
            digit = srp.tile([128, 1], F32, tag="digit")
            nc.vector.tensor_scalar(
                dscr[:], tot[:], float(CAPACITY), None, op0=Alu.is_ge
            )
            nc.vector.reduce_sum(digit[:], dscr[:], axis=AX.X)
            inc = srp.tile([128, 1], I32, tag="inc")
            nc.vector.tensor_scalar(inc[:], digit[:], float(base), None, op0=Alu.mult)
            nc.vector.tensor_tensor(tau[:], tau[:], inc[:], op=Alu.bitwise_or)

        # ---------------- exact stable tie-break ----------------
        mge = selp.tile([128, 128], F32)
        s2col = selp.tile([128, 2], F32)
        nc.vector.tensor_scalar(
            mge[:], pkey[:], tau[:].bitcast(F32), None, op0=Alu.is_ge
        )
        nc.vector.reduce_sum(s2col[:, 0:1], mge[:], axis=AX.X)
        meq = selp.tile([128, 128], F32)
        nc.vector.tensor_scalar(
            meq[:], pkey[:], tau[:].bitcast(F32), None, op0=Alu.is_equal
        )
        nc.vector.reduce_sum(s2col[:, 1:2], meq[:], axis=AX.X)
        tot2 = pss.tile([128, 2], F32, tag="tot2")
        nc.tensor.matmul(tot2[:], ones_mat[:], s2col[:], start=True, stop=True)
        tot2_sb = selp.tile([128, 2], F32)
        nc.vector.tensor_copy(tot2_sb[:], tot2[:])
        budget = selp.tile([128, 1], F32)
        # budget = CAP - (tot_ge - tot_eq)
        nc.vector.scalar_tensor_tensor(
            budget[:], in0=tot2_sb[:, 1:2], scalar=float(CAPACITY), in1=tot2_sb[:, 0:1],
            op0=Alu.add, op1=Alu.subtract,
        )
        pres = selp.tile([128, 128], F32)
        nc.vector.tensor_tensor_scan(
            pres[:], meq[:], meq[:], 0.0, op0=Alu.add, op1=Alu.bypass
        )
        offs = pss.tile([128, 1], F32, tag="offs")
        nc.tensor.matmul(offs[:], lmat[:], pres[:, 127:128], start=True, stop=True)
        offs_sb = selp.tile([128, 1], F32)
        nc.vector.tensor_copy(offs_sb[:], offs[:])
        excl = selp.tile([128, 128], F32)
        nc.vector.scalar_tensor_tensor(
            excl[:], in0=pres[:], scalar=offs_sb[:], in1=meq[:],
            op0=Alu.add, op1=Alu.subtract,
        )
        tieok = selp.tile([128, 128], F32)
        nc.vector.tensor_scalar(tieok[:], excl[:], budget[:], None, op0=Alu.is_lt)
        tkeep = selp.tile([128, 128], F32)
        nc.vector.tensor_tensor(tkeep[:], meq[:], tieok[:], op=Alu.mult)
        keep = selp.tile([128, 128], F32)
        nc.vector.tensor_tensor(keep[:], mge[:], meq[:], op=Alu.subtract)
        nc.vector.tensor_tensor(keep[:], keep[:], tkeep[:], op=Alu.add)
        nc.sync.dma_start(keep_out[:], keep[:])

        # ---------------- stats ----------------
        colsum_ps = pss.tile([N_EXPERTS, 1], F32, tag="colsum")
        nc.tensor.matmul(colsum_ps[:], colacc[:], ones_col[:], start=True, stop=True)
        colsum_sb = selp.tile([N_EXPERTS, 1], F32)
        nc.vector.tensor_copy(colsum_sb[:], colsum_ps[:])
        zrow = selp.tile([128, 1], F32)
        nc.vector.reduce_sum(zrow[:], zsq[:], axis=AX.X)
        zps = pss.tile([1, 1], F32, tag="zps")
        nc.tensor.matmul(zps[:], zrow[:], ones_col[:], start=True, stop=True)
        zsb = selp.tile([1, 1], F32)
        nc.vector.tensor_copy(zsb[:], zps[:])
        nc.sync.dma_start(
            stats_out[0:N_EXPERTS].rearrange("(p o) -> p o", o=1), colsum_sb[:]
        )
        nc.sync.dma_start(
            stats_out[N_EXPERTS : N_EXPERTS + 1].rearrange("(p o) -> p o", o=1),
            zsb[:],
        )


def build_program():
    nc = bacc.Bacc(
        "TRN2",
        target_bir_lowering=False,
        debug=False,
        enable_asserts=False,
        num_devices=N_CORES,
    )
    xt = nc.dram_tensor("xt", [D_MODEL, TPC], F32, kind="ExternalInput").ap()
    wt = nc.dram_tensor(
        "wt", [128, N_CHUNKS * N_EXPERTS], F32, kind="ExternalInput"
    ).ap()
    cid = nc.dram_tensor("cid", [128, 1], F32, kind="ExternalInput").ap()
    payload_out = nc.dram_tensor(
        "payload", [2, LOCAL_NK], F32, kind="ExternalOutput"
    ).ap()
    keep_out = nc.dram_tensor("keep", [128, 128], F32, kind="ExternalOutput").ap()
    stats_out = nc.dram_tensor("stats", [16], F32, kind="ExternalOutput").ap()

    with tile.TileContext(nc) as tc:
        _body(nc, tc, xt, wt, cid, payload_out, keep_out, stats_out)
    nc.compile()
    return nc


_PROGRAM = None


def _get_program():
    global _PROGRAM
    if _PROGRAM is None:
        _PROGRAM = build_program()
    return _PROGRAM


def make_in_maps(x, gate_weight):
    x = np.asarray(x, dtype=np.float32)
    gw = np.asarray(gate_weight, dtype=np.float32)
    # wt[p, c*8 + e] = gw[e, c*128 + p]
    wt = np.ascontiguousarray(
        gw.T.reshape(N_CHUNKS, 128, N_EXPERTS).transpose(1, 0, 2).reshape(128, -1)
    )
    in_maps = []
    t = np.arange(TPC)
    perm = 8 * (t & 127) + (t >> 7)  # device slot t_dev holds host token perm[t_dev]
    for c in range(N_CORES):
        xt_c = x[c * TPC : (c + 1) * TPC, :].T
        in_maps.append(
            {
                "xt": np.ascontiguousarray(xt_c[:, perm]),
                "wt": wt,
                "cid": np.full((128, 1), float(c), np.float32),
            }
        )
    return in_maps


def assemble_outputs(per_core):
    """per_core: list of dicts with payload [2,2048], keep [128,128], stats [16]."""
    idx_all = np.empty((N_TOKENS, TOP_K), np.int32)
    p_all = np.empty((N_TOKENS, TOP_K), np.float32)
    keep_planes = []
    colsum = np.zeros(N_EXPERTS, np.float64)
    zsum = 0.0
    for c in range(N_CORES):
        r = per_core[c]
        pay = r["payload"]
        p_all[c * TPC : (c + 1) * TPC] = pay[0].reshape(TPC, TOP_K)
        idx_all[c * TPC : (c + 1) * TPC] = np.rint(
            pay[1].reshape(TPC, TOP_K)
        ).astype(np.int32)
        keep_planes.append(r["keep"].reshape(-1))
        colsum += r["stats"][0:N_EXPERTS].astype(np.float64)
        zsum += float(r["stats"][N_EXPERTS])
    keep_flat = np.zeros(N_TOKENS * TOP_K, np.float32)
    for kp in keep_planes:
        keep_flat += kp
    keep = (keep_flat > 0.5).reshape(N_TOKENS, TOP_K)
    final_idx = np.where(keep, idx_all, -1).astype(np.int32)
    final_p = np.where(keep, p_all, 0.0).astype(np.float32)
    usage = np.array([kp[0::2].sum() for kp in keep_planes], np.float32)
    lb_loss = np.float32(
        (colsum * usage.astype(np.float64)).sum() * LB_WEIGHT / (N_TOKENS * N_EXPERTS)
    )
    z_loss = np.float32(zsum / N_TOKENS * Z_WEIGHT)
    return final_idx, final_p, lb_loss, z_loss, usage


def run_device(x, gate_weight, trace=False, **kwargs):
    nc = _get_program()
    in_maps = make_in_maps(x, gate_weight)
    res = bass_utils.run_bass_kernel_spmd(
        nc, in_maps, core_ids=list(range(N_CORES)), trace=trace, **kwargs
    )
    return res


def kernel(x, gate_weight):
    res = run_device(x, gate_weight, trace=False)
    return assemble_outputs(res.results)
